# revision 1
# baseline (speedup 1.0000x reference)
"""GCN encoder (3-layer GCNConv, normalize=False) on 8 Trainium2 NeuronCores.

Strategy (sharding_hint: shard nodes/dst-segments, all-gather src features,
replicate weights):
  - Nodes are packed into 32-node "blocks" (LPT-balanced by in-degree so every
    block has ~equal edge count), 400 blocks per core -> core c owns padded
    node rows [c*12800, (c+1)*12800).
  - Per layer:  h = x_local @ W_l   (PE matmul, per 128-node group with a PE
    transpose to build the stationary operand), written to a local DRAM
    buffer, then AllGather -> replicated h_full [102400, 128].
  - Aggregation: edges are packed (host-side) into tiles of 128 edges, all
    edges of a tile target one 32-node block.  For each group (4 blocks,
    16 tiles) one indirect-DMA gathers the 16*128 source rows from h_full
    into SBUF; a weighted one-hot selection matrix (built on DVE from the
    tile's dst offsets * edge weight) turns segment-sum into PE matmuls
    accumulated in PSUM (4 col-groups of 32 partitions).
  - bias + ReLU fused on DVE/ACT, feeding the next layer's transform without
    touching DRAM.  3 layers + 3 AllGathers in a single NEFF.
"""

import os
import sys
import heapq
import numpy as np

sys.path.insert(0, "/opt/trn_rl_repo")

N_NODES = 100000
N_EDGES = 1600000
D = 128
L = 3
C = 8  # cores
BLK = 32  # nodes per block (dst window of one selection matmul)
P = 128  # partitions / edge-tile size


class Cfg:
    def __init__(self, n_nodes, n_edges, b_pc, t_pb):
        self.n_nodes = n_nodes
        self.n_edges = n_edges
        self.b_pc = b_pc              # blocks per core
        self.t_pb = t_pb              # edge tiles (of 128) per block
        self.n_pc = b_pc * BLK        # padded nodes per core
        self.groups = b_pc // 4       # 128-node groups per core
        self.nt = b_pc * t_pb         # edge tiles per core
        assert b_pc % 4 == 0


def partition_graph(cfg, edge_src, edge_dst, edge_weight):
    """Assign nodes to 32-node blocks (balanced by in-degree), pack edges into
    128-edge tiles per block.  Returns per-core index/dst/weight arrays and the
    node -> padded-row mapping."""
    n, e = cfg.n_nodes, cfg.n_edges
    nb = C * cfg.b_pc
    cap_e = cfg.t_pb * P

    deg = np.bincount(edge_dst, minlength=n).astype(np.int64)
    order = np.argsort(-deg, kind="stable")

    block_of = np.empty(n, np.int64)
    slot_of = np.empty(n, np.int64)
    count = np.zeros(nb, np.int64)
    load = np.zeros(nb, np.int64)
    heap = [(0, b) for b in range(nb)]
    heapq.heapify(heap)
    for node in order:
        while True:
            ld, b = heapq.heappop(heap)
            if count[b] < BLK:
                break
        block_of[node] = b
        slot_of[node] = count[b]
        count[b] += 1
        load[b] = ld + deg[node]
        if count[b] < BLK:
            heapq.heappush(heap, (load[b], b))
    max_load = load.max()
    assert max_load <= cap_e, (
        f"block edge load {max_load} exceeds capacity {cap_e}; "
        f"raise t_pb to {int(np.ceil(max_load / P))}"
    )

    pid = block_of * BLK + slot_of  # padded global row id

    # pack edges: edge belongs to block of its dst
    eb = block_of[edge_dst]
    eord = np.argsort(eb, kind="stable")
    eb_s = eb[eord]
    starts = np.searchsorted(eb_s, np.arange(nb))
    rank = np.arange(e) - starts[eb_s]
    tile_local = rank // P
    part = rank % P
    core = eb_s // cfg.b_pc
    col = (eb_s % cfg.b_pc) * cfg.t_pb + tile_local

    idx_arr = np.zeros((C, P, cfg.nt), np.int32)
    dst_arr = np.zeros((C, P, cfg.nt), np.float32)
    w_arr = np.zeros((C, P, cfg.nt), np.float32)
    idx_arr[core, part, col] = pid[edge_src[eord]].astype(np.int32)
    dst_arr[core, part, col] = slot_of[edge_dst[eord]].astype(np.float32)
    w_arr[core, part, col] = edge_weight[eord].astype(np.float32)

    return idx_arr, dst_arr, w_arr, pid


def build_nc(cfg, repeats=1, n_swdge_queues=1, mm_interleave=True):
    import concourse.bacc as bacc
    import concourse.bass as bass
    import concourse.mybir as mybir
    import concourse.tile as tile

    f32 = mybir.dt.float32
    i32 = mybir.dt.int32
    n_pc, nt, groups, t_pb = cfg.n_pc, cfg.nt, cfg.groups, cfg.t_pb
    TPG = 4 * t_pb  # tiles per group

    nc = bacc.Bacc(
        "TRN2",
        target_bir_lowering=False,
        debug=False,
        num_devices=C,
        num_swdge_queues=n_swdge_queues,
    )

    x_in = nc.dram_tensor("x_in", [n_pc, D], f32, kind="ExternalInput")
    idx_in = nc.dram_tensor("idx_in", [P, nt], i32, kind="ExternalInput")
    dst_in = nc.dram_tensor("dst_in", [P, nt], f32, kind="ExternalInput")
    w_in = nc.dram_tensor("w_in", [P, nt], f32, kind="ExternalInput")
    W_in = nc.dram_tensor("W_in", [L, D, D], f32, kind="ExternalInput")
    b_in = nc.dram_tensor("b_in", [L, P, D], f32, kind="ExternalInput")  # replicated
    iota_in = nc.dram_tensor("iota_in", [P, BLK], f32, kind="ExternalInput")
    ident_in = nc.dram_tensor("ident_in", [P, P], f32, kind="ExternalInput")
    x_out = nc.dram_tensor("x_out", [n_pc, D], f32, kind="ExternalOutput")

    with tile.TileContext(nc) as tc:
        with (
            tc.tile_pool(name="const", bufs=1) as cpool,
            tc.tile_pool(name="work", bufs=3) as spool,
            tc.tile_pool(name="msgp", bufs=3) as mpool,
            tc.tile_pool(name="psum", bufs=2, space="PSUM") as ppool,
            tc.tile_pool(name="dram", bufs=1, space="DRAM") as dpool,
        ):
            # --- constants resident in SBUF ---
            idx_sb = cpool.tile([P, nt], i32, tag="idx")
            nc.sync.dma_start(out=idx_sb[:], in_=idx_in[:])
            dst_sb = cpool.tile([P, nt], f32, tag="dst")
            nc.sync.dma_start(out=dst_sb[:], in_=dst_in[:])
            w_sb = cpool.tile([P, nt], f32, tag="w")
            nc.sync.dma_start(out=w_sb[:], in_=w_in[:])
            W_sb = cpool.tile([P, L * D], f32, tag="W")
            for l in range(L):
                nc.sync.dma_start(out=W_sb[:, l * D:(l + 1) * D], in_=W_in[l])
            b_sb = cpool.tile([P, L * D], f32, tag="b")
            for l in range(L):
                nc.sync.dma_start(out=b_sb[:, l * D:(l + 1) * D], in_=b_in[l])
            iota_sb = cpool.tile([P, BLK], f32, tag="iota")
            nc.sync.dma_start(out=iota_sb[:], in_=iota_in[:])
            id_sb = cpool.tile([P, P], f32, tag="ident")
            nc.sync.dma_start(out=id_sb[:], in_=ident_in[:])

            h_loc = [
                dpool.tile([n_pc, D], f32, tag=f"hloc{l}", name=f"hloc{l}")
                for l in range(L)
            ]
            # Local (per-core replicated) AllGather output: reads are local-HBM.
            # A "Shared" output would make every gather a remote-HBM read
            # (~100x slower per indirect call, measured).
            h_full = [
                dpool.tile([C * n_pc, D], f32, tag=f"hfull{l}", name=f"hfull{l}")
                for l in range(L)
            ]

            def transform(l, g, x_blk):
                """h_loc[l][g*128:(g+1)*128] = x_blk @ W_l  (x_blk: SBUF [128,128])"""
                psum_t = ppool.tile([P, P], f32, tag="psum_t")
                nc.tensor.transpose(out=psum_t[:], in_=x_blk[:], identity=id_sb[:])
                xT = spool.tile([P, P], f32, tag="xT")
                nc.vector.tensor_copy(out=xT[:], in_=psum_t[:])
                psum_h = ppool.tile([P, P], f32, tag="psum_h")
                nc.tensor.matmul(
                    out=psum_h[:],
                    lhsT=xT[:],
                    rhs=W_sb[:, l * D:(l + 1) * D],
                    start=True,
                    stop=True,
                )
                h_sb = spool.tile([P, P], f32, tag="h_sb")
                nc.vector.tensor_copy(out=h_sb[:], in_=psum_h[:])
                nc.sync.dma_start(
                    out=h_loc[l][g * P:(g + 1) * P, :], in_=h_sb[:]
                )

            def all_gather(l):
                nc.gpsimd.collective_compute(
                    "AllGather",
                    mybir.AluOpType.bypass,
                    replica_groups=[list(range(C))],
                    ins=[h_loc[l][:, :]],
                    outs=[h_full[l][:, :]],
                )

            def aggregate(l, g):
                """Return SBUF tile [128,128] = relu(segsum(w*h_full[src]) + b_l)
                for the 128 dst nodes of group g."""
                msg = mpool.tile([P, TPG * P], f32, tag="msg")
                # HW indirect DMA consumes exactly one index per partition:
                # one call per 128-edge tile, round-robined over SWDGE queues.
                for ti in range(TPG):
                    call = nc.gpsimd.indirect_dma_start(
                        out=msg[:, ti * P:(ti + 1) * P],
                        out_offset=None,
                        in_=h_full[l][:, :],
                        in_offset=bass.IndirectOffsetOnAxis(
                            ap=idx_sb[:, g * TPG + ti:g * TPG + ti + 1], axis=0
                        ),
                    )
                    qi = ti % n_swdge_queues
                    if qi:
                        call.ins.queue = f"qPoolDynamic{qi}"
                sel = spool.tile([P, TPG * BLK], f32, tag="sel")
                sel3 = sel[:].rearrange("p (t j) -> p t j", j=BLK)
                dst3 = dst_sb[:, g * TPG:(g + 1) * TPG].to_broadcast([P, TPG, BLK])
                w3 = w_sb[:, g * TPG:(g + 1) * TPG].to_broadcast([P, TPG, BLK])
                iota3 = (
                    iota_sb[:]
                    .rearrange("p (o j) -> p o j", o=1)
                    .to_broadcast([P, TPG, BLK])
                )
                nc.vector.tensor_tensor(
                    out=sel3, in0=dst3, in1=iota3, op=mybir.AluOpType.is_equal
                )
                nc.vector.tensor_tensor(
                    out=sel3, in0=sel3, in1=w3, op=mybir.AluOpType.mult
                )
                psum_agg = ppool.tile([P, P], f32, tag="psum_agg")
                # mm_interleave: consecutive matmuls hit different PE column
                # groups (32-col subarrays run concurrently)
                order = (
                    [(t, w) for t in range(t_pb) for w in range(4)]
                    if mm_interleave
                    else [(t, w) for w in range(4) for t in range(t_pb)]
                )
                for t, w in order:
                    ti = w * t_pb + t
                    nc.tensor.matmul(
                        out=psum_agg[w * BLK:(w + 1) * BLK, :],
                        lhsT=sel[:, ti * BLK:(ti + 1) * BLK],
                        rhs=msg[:, ti * P:(ti + 1) * P],
                        start=(t == 0),
                        stop=(t == t_pb - 1),
                        tile_position=(0, w * BLK),
                    )
                xnew = spool.tile([P, P], f32, tag="xnew")
                nc.vector.tensor_tensor(
                    out=xnew[:],
                    in0=psum_agg[:],
                    in1=b_sb[:, l * D:(l + 1) * D],
                    op=mybir.AluOpType.add,
                )
                nc.vector.tensor_scalar_max(xnew[:], xnew[:], 0.0)
                return xnew

            # layer 0 transform from the input
            for _rep in range(repeats):  # >1 only for timing calibration
                for g in range(groups):
                    x_blk = spool.tile([P, P], f32, tag="x_blk")
                    nc.sync.dma_start(out=x_blk[:], in_=x_in[g * P:(g + 1) * P, :])
                    transform(0, g, x_blk)
                all_gather(0)
                for l in range(L):
                    last = l == L - 1
                    for g in range(groups):
                        xnew = aggregate(l, g)
                        if last:
                            nc.sync.dma_start(
                                out=x_out[g * P:(g + 1) * P, :], in_=xnew[:]
                            )
                        else:
                            transform(l + 1, g, xnew)
                    if not last:
                        all_gather(l + 1)

    nc.compile()
    return nc


def make_host_inputs(cfg, x, edge_src, edge_dst, edge_weight, W, b):
    idx_arr, dst_arr, w_arr, pid = partition_graph(cfg, edge_src, edge_dst, edge_weight)
    x_sh = np.zeros((C, cfg.n_pc, D), np.float32)
    x_sh.reshape(C * cfg.n_pc, D)[pid] = x
    b_rep = np.broadcast_to(b[:, None, :], (L, P, D)).astype(np.float32).copy()
    iota = np.broadcast_to(np.arange(BLK, dtype=np.float32), (P, BLK)).copy()
    ident = np.eye(P, dtype=np.float32)
    in_maps = [
        {
            "x_in": x_sh[c],
            "idx_in": idx_arr[c],
            "dst_in": dst_arr[c],
            "w_in": w_arr[c],
            "W_in": np.asarray(W, np.float32),
            "b_in": b_rep,
            "iota_in": iota,
            "ident_in": ident,
        }
        for c in range(C)
    ]
    return in_maps, pid


def unshard_output(cfg, results, pid):
    full = np.concatenate([results[c]["x_out"] for c in range(C)], axis=0)
    return full[pid]


def run(x, edge_src, edge_dst, edge_weight, W, b, trace=False, trace_kwargs=None):
    from concourse.bass_utils import run_bass_kernel_spmd

    cfg = Cfg(N_NODES, N_EDGES, b_pc=400, t_pb=4)
    x = np.asarray(x, np.float32)
    edge_src = np.asarray(edge_src, np.int32)
    edge_dst = np.asarray(edge_dst, np.int32)
    edge_weight = np.asarray(edge_weight, np.float32)
    in_maps, pid = make_host_inputs(cfg, x, edge_src, edge_dst, edge_weight, W, b)
    nc = build_nc(cfg)
    res = run_bass_kernel_spmd(
        nc,
        in_maps,
        core_ids=list(range(C)),
        trace=trace,
        **(trace_kwargs or {}),
    )
    return unshard_output(cfg, res.results, pid), res


def kernel(x, edge_src, edge_dst, edge_weight, W, b):
    out, _ = run(x, edge_src, edge_dst, edge_weight, W, b)
    return out

